# revision 9
# baseline (speedup 1.0000x reference)
"""Discretized-mixture NLL loss kernel for Trainium2 (Bass/Tile), 8-core data parallel.

Math per pixel/channel c, mixtures m=0..9 (matches reference):
    xhat = W @ px + b            (1x1 conv, 90 outputs = [pi(30) | mu(30) | ls(30)])
    s2 = (1/sigma)/sqrt2 = exp(-8*tanh(ls/8) + ln(1/sqrt2));  D = mu - xe
    dcdf = 0.5*(erf((D+d)*s2) - erf((D-d)*s2))
    num  = 0.5*sum_m g_m*dlt_m + eps*den ; den = sum_m g_m ; nll = ln(den) - ln(num)

v3: bf16 datapath. Host casts px_z/x/W to bf16 (halves HBM traffic; GEMM
accumulates fp32 in PSUM; max rel err ~4e-3 vs 2e-2 gate). Per core nb=4
images, 16 supertiles of 1024 px, 2 groups of 8.
  - loads: [128, 4096] bf16 tiles (8KB/partition descriptors), one per (img, k).
  - GEMM per supertile: psum [128, 1024], 8 bf16 MMs (wt k-chunks, weight pairs
    back-to-back) + 2 bias MMs (K=4 rows carry -xe and b; xq held as one
    [128, 4096] tile, image i at partition 32i).
  - ScalarE: tanh -> ps[96:128]; exp [64,1024] -> s2g bf16 [g | s2] (per-row
    scale/bias). Erf on [128, 2048] group stacks (table phases: A,B,A,B,Ln).
  - VectorE: STT hi/lo -> group stacks; dlt = elo-ehi [128,2048] bf16;
    qd = g*dlt overwrites s2 rows (all-bf16).
  - Reduction: per subtile 2 bf16 MMs (num via l1n: 0.5*qd + eps*g; den via
    l1d) into span-stacked psum [128,1024] (rows 32*s4). Copies (scalar for
    num, vector for den) -> packed [128, 4096] bf16; one Ln per half; nll =
    column-offset subtract; [128, 2048] f32 out DMA (host drops pad rows).
"""

import numpy as np
from ml_dtypes import bfloat16

WIDTH = 512
C_IMG = 3
N_MIX = 10
SIZE = 64
STD = 127.5
EPS = 1e-8
DELTA = 1.0 / STD / 2.0
LOG_INV_SQRT2 = -0.34657359027997264
N_CORES = 8
SUP_W = 1024          # pixels per supertile
SUB_W = 512           # matmul moving-dim tile
GRP = 8               # supertiles per activation-table group


def make_consts(W, b):
    """Host-side prep of the small constant tensors (32-padded blocks)."""
    W = np.asarray(W, np.float32)
    b = np.asarray(b, np.float32)
    # lhsT column blocks, M=128 (enables FWL), two variants:
    #   A: [mu(0:30) | ls(32:62) | pi(64:94) | Z(96:128)]  -> g lands at base 0
    #   B: [mu(0:30) | ls(32:62) | Z(64:96) | pi(96:126)]  -> g lands at base 32
    # Alternating variants by supertile parity aligns g with the dlt band base
    # so the qd TensorTensor has equal input base partitions.
    wt = np.zeros((2, WIDTH, 128), np.float32)
    bx = np.zeros((2, 4, 128), np.float32)      # K=4 rows: (xe0, xe1, xe2, ones)
    for v, pio in ((0, 64), (1, 96)):
        wt[v, :, 0:30] = W[30:60].T             # mu
        wt[v, :, 32:62] = W[60:90].T            # logsigma
        wt[v, :, pio:pio + 30] = W[0:30].T      # pi logits
        bx[v, 3, 0:30] = b[30:60]
        bx[v, 3, 32:62] = b[60:90]
        bx[v, 3, pio:pio + 30] = b[0:30]
        for r in range(30):
            bx[v, r % 3, r] = -1.0              # D rows get -xe_c
    wt = np.ascontiguousarray(wt).astype(bfloat16)
    bx = np.ascontiguousarray(bx).astype(bfloat16)
    # reduction lhsTs. s2g after qd: variant A = [g(0:32) | qd(32:64)],
    # variant B = [qd(0:32) | g(32:64)]. M=64 even/odd column-half variants so
    # two subtiles accumulate into one [64, 512] psum region (out base limited
    # to 0/32/64); subtile parity == weight-layout variant.
    l1n = np.zeros((2, 64, 64), np.float32)     # num' = 0.5*sum qd + eps*den
    l1d = np.zeros((2, 64, 64), np.float32)     # den = sum g
    for e in range(2):
        o = 32 * e                              # output column half
        gr, qr = (0, 32) if e == 0 else (32, 0)  # g/qd row blocks for variant e
        for r in range(30):
            c = r % 3
            l1n[e, gr + r, o + c] = EPS
            l1n[e, qr + r, o + c] = 0.5
            l1d[e, gr + r, o + c] = 1.0
        for v in range(o + 3, o + 32):
            l1n[e, gr:gr + 30, v] = 1.0         # dummies: den-like, Ln finite
            l1d[e, gr:gr + 30, v] = 1.0
    l1n = l1n.astype(bfloat16)
    l1d = l1d.astype(bfloat16)
    scb = np.zeros((2, 64, 2), np.float32)      # merged-exp (scale, bias) rows
    for v in range(2):
        gr, sr = (0, 32) if v == 0 else (32, 0)
        scb[v, gr:gr + 32, 0] = 1.0             # pi rows: exp(x) = g
        scb[v, sr:sr + 32, 0] = -8.0            # tanh rows: exp(-8*t + c) = s2
        scb[v, sr:sr + 32, 1] = LOG_INV_SQRT2
    return wt, bx, l1n, l1d, scb


def build_nc(n_batch=4):
    """Build the single-core Bass program (same NEFF runs SPMD on all cores)."""
    from contextlib import ExitStack

    import concourse.bacc as bacc
    import concourse.mybir as mybir
    import concourse.tile as tile
    from concourse.tile import add_dep_helper

    f32 = mybir.dt.float32
    bf16 = mybir.dt.bfloat16
    ALU = mybir.AluOpType
    ACT = mybir.ActivationFunctionType

    PX_IMG = SIZE * SIZE                        # 4096
    S = n_batch * PX_IMG // SUP_W               # supertiles per core (16)
    assert S % GRP == 0
    n_grp = S // GRP                            # 2

    nc = bacc.Bacc("TRN2", target_bir_lowering=False, debug=False)
    pz = nc.dram_tensor("pz", [n_batch, WIDTH, PX_IMG], bf16, kind="ExternalInput").ap()
    x4 = nc.dram_tensor("x4", [S, 4, SUP_W], bf16, kind="ExternalInput").ap()
    wt = nc.dram_tensor("wt", [2, WIDTH, 128], bf16, kind="ExternalInput").ap()
    bx = nc.dram_tensor("bx", [2, 4, 128], bf16, kind="ExternalInput").ap()
    l1n = nc.dram_tensor("l1n", [2, 64, 64], bf16, kind="ExternalInput").ap()
    l1d = nc.dram_tensor("l1d", [2, 64, 64], bf16, kind="ExternalInput").ap()
    scb = nc.dram_tensor("scb", [2, 64, 2], f32, kind="ExternalInput").ap()
    out = nc.dram_tensor("out", [128, 2048 * n_grp], f32, kind="ExternalOutput").ap()

    with tile.TileContext(nc) as tc, ExitStack() as ctx:
        const_pool = ctx.enter_context(tc.tile_pool(name="const", bufs=1))
        xt_pool = ctx.enter_context(tc.tile_pool(name="xt", bufs=2))
        xq_pool = ctx.enter_context(tc.tile_pool(name="xq", bufs=3))
        hl_pool = ctx.enter_context(tc.tile_pool(name="hl", bufs=2))
        e_pool = ctx.enter_context(tc.tile_pool(name="e", bufs=1))
        s2g_pool = ctx.enter_context(tc.tile_pool(name="s2g", bufs=2 * GRP + 1))
        dlt_pool = ctx.enter_context(tc.tile_pool(name="dlt", bufs=1))
        tail_pool = ctx.enter_context(tc.tile_pool(name="tail", bufs=1))
        ln_pool = ctx.enter_context(tc.tile_pool(name="ln", bufs=1))
        nll_pool = ctx.enter_context(tc.tile_pool(name="nll", bufs=1))
        ps_pool = ctx.enter_context(tc.tile_pool(name="ps", bufs=2, space="PSUM"))
        psn_pool = ctx.enter_context(tc.tile_pool(name="psn", bufs=1, space="PSUM"))
        psd_pool = ctx.enter_context(tc.tile_pool(name="psd", bufs=1, space="PSUM"))

        # --- constants ---
        wt_sb = const_pool.tile([128, 2 * 4 * 128], bf16)
        nc.sync.dma_start(
            wt_sb[:].rearrange("i (v k o) -> i v k o", v=2, o=128),
            wt.rearrange("v (k i) o -> i v k o", i=128),
        )
        bx_sb = const_pool.tile([4, 2 * 128], bf16)
        nc.sync.dma_start(bx_sb[:].rearrange("p (v o) -> p v o", v=2),
                          bx.rearrange("v p o -> p v o"))
        l1n_sb = const_pool.tile([64, 2 * 64], bf16)
        nc.sync.dma_start(l1n_sb[:].rearrange("p (e o) -> p e o", e=2),
                          l1n.rearrange("e p o -> p e o"))
        l1d_sb = const_pool.tile([64, 2 * 64], bf16)
        nc.sync.dma_start(l1d_sb[:].rearrange("p (e o) -> p e o", e=2),
                          l1d.rearrange("e p o -> p e o"))
        scb_sb = const_pool.tile([64, 4], f32)
        nc.sync.dma_start(scb_sb[:].rearrange("p (v c) -> p v c", v=2),
                          scb.rearrange("v p c -> p v c"))


        packed = [tail_pool.tile([128, 4096], bf16, tag=f"packed{g}", name=f"packed{g}")
                  for g in range(n_grp)]

        # ACT table-set ordering chain (phases: A=tanh/exp, B=erf, C=ln)
        act_chain = []

        def chain(inst):
            if act_chain:
                add_dep_helper(inst.ins, act_chain[-1].ins, sync=False,
                               reason="act table-set batching")
            act_chain.append(inst)
            return inst

        xts = {}

        def load_image(img):
            ts = []
            for k in range(4):
                t = xt_pool.tile([128, PX_IMG], bf16, tag=f"xt{k}")
                nc.sync.dma_start(t[:], pz[img, 128 * k:128 * (k + 1), :])
                ts.append(t)
            xts[img] = ts

        def phase1(sup, hi_t, lo_t):
            img, col = divmod(sup, 4)
            jj = sup % 4                        # row block in hi/lo stack
            q = (sup % GRP) // 4                # column block in hi/lo stack
            if col == 0 and img not in xts:
                load_image(img)
            xt = xts[img]
            xq_t = xq_pool.tile([4, SUP_W], bf16, tag="xq")
            nc.sync.dma_start(xq_t[:], x4[sup])
            vv = jj % 2                         # weight-layout variant
            ps = ps_pool.tile([128, SUP_W], f32, tag="ps")
            for k in range(4):
                for t in range(2):
                    sl = slice(SUP_W * col + SUB_W * t, SUP_W * col + SUB_W * (t + 1))
                    nc.tensor.matmul(
                        ps[:, SUB_W * t:SUB_W * (t + 1)],
                        wt_sb[:, 128 * (4 * vv + k):128 * (4 * vv + k + 1)],
                        xt[k][:, sl],
                        start=(k == 0), stop=False,
                    )
            for t in range(2):
                nc.tensor.matmul(
                    ps[:, SUB_W * t:SUB_W * (t + 1)],
                    bx_sb[:, 128 * vv:128 * (vv + 1)],
                    xq_t[:, SUB_W * t:SUB_W * (t + 1)],
                    start=False, stop=True,
                )
            tb = 96 if vv == 0 else 64          # tanh lands in the Z block
            chain(nc.scalar.activation(ps[tb:tb + 32, :], ps[32:64, :], ACT.Tanh, scale=0.125))
            s2g_t = s2g_pool.tile([64, SUP_W], bf16, tag="s2g")
            chain(nc.scalar.activation(
                s2g_t[:], ps[64:128, :], ACT.Exp,
                bias=scb_sb[:, 2 * vv + 1:2 * vv + 2], scale=scb_sb[:, 2 * vv:2 * vv + 1],
            ))
            hb = 32 * jj
            cb = SUP_W * q
            sr = 32 * (1 - vv)                  # s2 rows: A at 32:64, B at 0:32
            nc.vector.scalar_tensor_tensor(
                hi_t[hb:hb + 32, cb:cb + SUP_W], ps[0:32, :], DELTA,
                s2g_t[sr:sr + 32, :], ALU.subtract, ALU.mult,
            )
            nc.vector.scalar_tensor_tensor(
                lo_t[hb:hb + 32, cb:cb + SUP_W], ps[0:32, :], DELTA,
                s2g_t[sr:sr + 32, :], ALU.add, ALU.mult,
            )
            return s2g_t

        def phase2(g, s2gs, dlt_t):
            # mixture reduction: per span of 4 supertiles, row-stacked psum
            for sp2 in range(2):
                psn_t = psn_pool.tile([128, SUP_W], f32, tag="psn")
                psd_t = psd_pool.tile([128, SUP_W], f32, tag="psd")
                for s4 in range(4):
                    j = 4 * sp2 + s4
                    s2g_t = s2gs[j]
                    vv = s4 % 2
                    h2 = s4 // 2
                    gb = 32 * vv                # g rows; qd overwrites s2 rows
                    nc.vector.tensor_tensor(
                        s2g_t[32 - gb:64 - gb, :], s2g_t[gb:gb + 32, :],
                        dlt_t[gb:gb + 32,
                              2048 * h2 + SUP_W * sp2:2048 * h2 + SUP_W * (sp2 + 1)],
                        ALU.mult,
                    )
                for p in range(2):
                    for t in range(2):
                        sl = slice(SUB_W * t, SUB_W * (t + 1))
                        for e in range(2):
                            s2g_t = s2gs[4 * sp2 + 2 * p + e]
                            nc.tensor.matmul(
                                psn_t[64 * p:64 * p + 64, sl],
                                l1n_sb[:, 64 * e:64 * e + 64], s2g_t[:, sl],
                                start=(e == 0), stop=(e == 1))
                        for e in range(2):
                            s2g_t = s2gs[4 * sp2 + 2 * p + e]
                            nc.tensor.matmul(
                                psd_t[64 * p:64 * p + 64, sl],
                                l1d_sb[:, 64 * e:64 * e + 64], s2g_t[:, sl],
                                start=(e == 0), stop=(e == 1))
                chain(nc.scalar.copy(
                    packed[g][:, SUP_W * sp2:SUP_W * (sp2 + 1)], psn_t[:]))
                nc.vector.tensor_scalar_mul(
                    packed[g][:, 2048 + SUP_W * sp2:2048 + SUP_W * (sp2 + 1)],
                    psd_t[:], 1.0)

        # prefetch first image, then run groups with erf of g overlapping
        # phase1 of g+1 (PE keeps streaming; table phases stay A,B,A,B,...,Ln)
        load_image(0)
        prev = None                              # (g, s2gs, dlt_t) pending phase2
        for g in range(n_grp):
            hi_t = hl_pool.tile([128, 2 * SUP_W], f32, tag="hi", name=f"hi{g}")
            lo_t = hl_pool.tile([128, 2 * SUP_W], f32, tag="lo", name=f"lo{g}")
            s2gs = [phase1(GRP * g + j, hi_t, lo_t) for j in range(GRP)]
            ehi_t = e_pool.tile([128, 2 * SUP_W], f32, tag="ehi", name=f"ehi{g}")
            elo_t = e_pool.tile([128, 2 * SUP_W], f32, tag="elo", name=f"elo{g}")
            chain(nc.scalar.activation(ehi_t[:], hi_t[:], ACT.Erf))
            chain(nc.scalar.activation(elo_t[:], lo_t[:], ACT.Erf))
            dlt_t = dlt_pool.tile([64, 4 * SUP_W], bf16, tag="dlt")
            for h2 in range(2):
                nc.vector.tensor_tensor(
                    dlt_t[:, 2048 * h2:2048 * (h2 + 1)],
                    elo_t[64 * h2:64 * (h2 + 1), :],
                    ehi_t[64 * h2:64 * (h2 + 1), :], ALU.subtract)
            if prev is not None:
                phase2(*prev)
            prev = (g, s2gs, dlt_t)
        phase2(*prev)

        # --- tail: one Ln per half, column-offset subtract, full-row DMA out ---
        for h in range(n_grp):
            ln_t = ln_pool.tile([128, 4096], f32, tag="ln")
            chain(nc.scalar.activation(ln_t[:], packed[h][:], ACT.Ln))
            nll_t = nll_pool.tile([128, 2048], f32, tag="nll")
            nc.vector.tensor_tensor(nll_t[:], ln_t[:, 2048:4096],
                                    ln_t[:, 0:2048], ALU.subtract)
            nc.sync.dma_start(out[:, 2048 * h:2048 * (h + 1)], nll_t[:])

    nc.compile()
    return nc


def prep_core_inputs(px_z_shard, x_shard, consts):
    """px_z_shard [nb, 512, 64, 64] f32, x_shard [nb, 64, 64, 3] f32 -> input map."""
    wt, bx, l1n, l1d, scb = consts
    nb = px_z_shard.shape[0]
    pzs = np.ascontiguousarray(
        px_z_shard.reshape(nb, WIDTH, SIZE * SIZE)).astype(bfloat16)
    S = nb * (SIZE * SIZE) // SUP_W
    xf = x_shard.reshape(S, SUP_W, C_IMG)
    x4 = np.ones((S, 4, SUP_W), np.float32)
    x4[:, 0:3, :] = xf.transpose(0, 2, 1)
    return {
        "pz": pzs, "x4": x4.astype(bfloat16), "wt": wt, "bx": bx,
        "l1n": l1n, "l1d": l1d, "scb": scb,
    }


def gather_core_output(o, nb):
    """o [128, 4096] f32 (row 32*s4+v, col (h, sp2, t, px)) -> [nb, 64, 64, 3]."""
    n_grp = nb * (SIZE * SIZE) // SUP_W
    n_grp //= GRP
    o6 = o.reshape(4, 32, n_grp, 2, 2, SUB_W)[:, 0:3]      # s4, c, h, sp2, t, px
    # supertile = 8h + 4*sp2 + s4 ; pixel = 1024*sup + 512*t + px
    o6 = o6.transpose(2, 3, 0, 4, 5, 1)                     # h, sp2, s4, t, px, c
    return np.ascontiguousarray(o6).reshape(nb, SIZE, SIZE, C_IMG)


_NC_CACHE = {}


def kernel(px_z, x, W, b):
    from concourse.bass_utils import run_bass_kernel_spmd

    px_z = np.asarray(px_z, np.float32)
    x = np.asarray(x, np.float32)
    B = px_z.shape[0]
    nb = B // N_CORES
    consts = make_consts(W, b)
    key = (nb,)
    if key not in _NC_CACHE:
        _NC_CACHE[key] = build_nc(n_batch=nb)
    nc = _NC_CACHE[key]
    in_maps = [
        prep_core_inputs(px_z[nb * i:nb * (i + 1)], x[nb * i:nb * (i + 1)], consts)
        for i in range(N_CORES)
    ]
    res = run_bass_kernel_spmd(nc, in_maps, core_ids=list(range(N_CORES)))
    outs = [gather_core_output(res.results[i]["out"], nb) for i in range(N_CORES)]
    return np.concatenate(outs, 0)
